# revision 27
# baseline (speedup 1.0000x reference)
"""DGANet dual-GAT layer on 8 Trainium2 NeuronCores (Bass/Tile).

Math (per branch b in {n, d}):
    Wh = h @ W_b                                  [4096, 256]
    e  = leaky_relu(s1_i + s2_j, 0.2)             s1 = h@(W@a1), s2 = h@(W@a2)
    att = softmax(where(adj>0, e, -9e15), axis=-1)
    f_b = elu(att @ Wh)
Output: f_n + f_d.

Sharding: 1D row-parallel over the 4096 attention rows (512 rows/core).
Each core computes the full Wh tile-by-tile (replicated; cheaper than any
collective under this interconnect) and holds its score block transposed,
P^T[j, i] (j on partitions).  The adjacency mask is a host-prepared additive
bias (0 or -16384, bf16): exp underflows masked entries to exactly 0.

The att @ Wh contraction runs with P^T chunks as the *stationary* operand
and Wh as the moving operand, so the output lands directly in [i, f] layout
(no final transpose) and the softmax denominator is a 1-column ones matmul
riding the same stationary load (~free).  Engine split per j-tile pair:
DVE does the masked-logit stt, Pool (otherwise idle) the leaky-relu stt and
the Wh PSUM->SBUF copies, Act the exp.  Whole data path bf16; PSUM f32.
W, W@a1, W@a2 are packed host-side into one bf16 [FIN, 258] operand per
branch so Wh, s2 and the s1 seed fall out of the same produce matmul.
"""

from contextlib import ExitStack

import numpy as np
import ml_dtypes

import concourse.bass as bass
import concourse.bacc as bacc
import concourse.mybir as mybir
import concourse.tile as tile
from concourse import bass_utils

N, FIN, F = 4096, 512, 256
NCORES = 8
R = N // NCORES            # 512 attention rows per core
P = 128                    # partitions
NJT = N // P               # 32 j-tiles
NKT = FIN // P             # 4 fin contraction tiles
NIC = R // P               # 4 i-chunks (output row blocks per core)
WC = F + 2                 # aug cols: [W | W@a2 | W@a1]
PC = F + 1                 # produce stream cols: [W | W@a2]
MASKB = -16384.0           # additive mask: exp(x - 16384) == 0.0
ALPHA = 0.2

F32 = mybir.dt.float32
BF16 = mybir.dt.bfloat16
FP8 = mybir.dt.float8e5
AF = mybir.ActivationFunctionType
ALU = mybir.AluOpType
BR = ("n", "d")

CH = 4                     # hT column chunks (DMA pipelining granularity)
CHW = N // CH              # 1024 cols per chunk
NPR = NJT // 2             # j-tile pairs per branch
DELAY = 9                  # produce/consume software pipelining depth
LAMK = 3                   # (unused) kept for reference


def is_lam(tp):
    """~56% of pairs run the DVE-heavy stt scheme (fp8 masks); the rest
    run the Act-heavy Prelu-bias scheme (bf16 masks, needed for 2x tt).
    Interleaved so neither engine sees clustered back-to-back heavy pairs."""
    return tp % 2 == 0 or tp % 8 == 1


def build_program(reps=None):
    """reps=None: single-shot program (grading path).  reps=K: body wrapped
    in a K-iteration hardware loop, for wall-clock HW timing by slope."""
    nc = bacc.Bacc("TRN2", target_bir_lowering=False, debug=False,
                   num_devices=NCORES)

    hT = nc.dram_tensor("ht", [FIN, N], BF16, kind="ExternalInput").ap()
    hTo = nc.dram_tensor("hto", [FIN, R], BF16, kind="ExternalInput").ap()
    WAUG = {b: nc.dram_tensor(f"waug_{b}", [FIN, WC], BF16,
                              kind="ExternalInput").ap()
            for b in BR}
    MT = {b: nc.dram_tensor(f"mt_{b}", [N, R], BF16, kind="ExternalInput").ap()
          for b in BR}
    MT8 = {b: nc.dram_tensor(f"mt8_{b}", [N, R], FP8,
                             kind="ExternalInput").ap()
           for b in BR}
    OUT = nc.dram_tensor("out", [R, F], F32, kind="ExternalOutput").ap()

    with tile.TileContext(nc) as tc:
        if reps is None:
            with ExitStack() as ctx:
                _body(ctx, nc, tc, hT, hTo, WAUG, MT, MT8, OUT)
        else:
            with tc.For_i(0, reps, 1,
                          hint_engines=(mybir.EngineType.PE,)):
                with ExitStack() as ctx:
                    _body(ctx, nc, tc, hT, hTo, WAUG, MT, MT8, OUT)
    nc.compile()
    return nc


def _body(ctx, nc, tc, hT, hTo, WAUG, MT, MT8, OUT):
    consts = ctx.enter_context(tc.tile_pool(name="consts", bufs=1))
    # produce psum tiles (also s1 seed + warmup), recycled
    pp_work = ctx.enter_context(tc.tile_pool(name="pp_work", bufs=3,
                                             space="PSUM"))
    # accumulators live across each branch: 4x att@Wh [i,f] + row sums
    pp_acc = ctx.enter_context(tc.tile_pool(name="pp_acc", bufs=1,
                                            space="PSUM"))
    whp = ctx.enter_context(tc.tile_pool(name="whp", bufs=12))
    maskp = ctx.enter_context(tc.tile_pool(name="maskp", bufs=12))
    workp = ctx.enter_context(tc.tile_pool(name="workp", bufs=4))
    pexp = ctx.enter_context(tc.tile_pool(name="pexp", bufs=12))
    epip = ctx.enter_context(tc.tile_pool(name="epip", bufs=2))

    ones16 = consts.tile([P, P], BF16, tag="ones16")
    nc.vector.memset(ones16, 1.0)

    # ---- weights first: s1 + produce(0) are the pipeline's critical path --
    waug_sb = {}
    for k in range(NKT):
        t = consts.tile([P, WC], BF16, tag=f"aug_n{k}")
        nc.sync.dma_start(out=t, in_=WAUG["n"][k * P:(k + 1) * P, :])
        waug_sb["n", k] = t

    # ---- own-row h block ---------------------------------------------------
    hto_sb = []
    for k in range(NKT):
        t = consts.tile([P, R], BF16, tag=f"hto{k}")
        nc.sync.dma_start(out=t, in_=hTo[k * P:(k + 1) * P, :])
        hto_sb.append(t)

    # ---- first hT chunk ----------------------------------------------------
    ht_sb = {}
    for k in range(NKT):
        t = consts.tile([P, CHW], BF16, tag=f"ht{k}_0")
        nc.sync.dma_start(out=t, in_=hT[k * P:(k + 1) * P, 0:CHW])
        ht_sb[k, 0] = t

    for k in range(NKT):
        t = consts.tile([P, WC], BF16, tag=f"aug_d{k}")
        nc.sync.dma_start(out=t, in_=WAUG["d"][k * P:(k + 1) * P, :])
        waug_sb["d", k] = t

    # PE warm-up: junk matmuls on resident constants so the HAM clock gate
    # ramps while the hT DMAs are still streaming.
    wps = pp_work.tile([P, P], F32, tag="pswork", name="wps")
    for _ in range(12):
        nc.tensor.matmul(wps, lhsT=ones16, rhs=ones16, start=True, stop=True)

    # wa1 chunk k replicated across 128 cols: stationary operand whose
    # matmul output is s1 already broadcast over partitions.
    wa_r = {}
    for b in BR:
        reps = []
        for k in range(NKT):
            r = consts.tile([P, P], BF16, tag=f"war_{b}{k}", name=f"war{k}")
            nc.vector.tensor_copy(
                out=r, in_=waug_sb[b, k][:, F + 1:F + 2].broadcast_to((P, P)))
            reps.append(r)
        wa_r[b] = reps

    # ---- remaining hT chunks + branch-n mask prefetch, interleaved -------
    mtiles = {}

    def issue_mask(b, tp):
        jt0 = 2 * tp
        if is_lam(tp):
            m = maskp.tile([P, 2 * R], FP8, tag="mask8", name="m8", bufs=9)
            src = MT8[b]
        else:
            m = maskp.tile([P, 2 * R], BF16, tag="mask", name="m", bufs=7)
            src = MT[b]
        nc.sync.dma_start(
            out=m.rearrange("p (two r) -> p two r", two=2),
            in_=src[jt0 * P:(jt0 + 2) * P, :].rearrange(
                "(two p) r -> p two r", two=2))
        mtiles[b, tp] = m

    for tp in range(4):
        issue_mask("n", tp)
    for ch in range(1, CH):
        for k in range(NKT):
            t = consts.tile([P, CHW], BF16, tag=f"ht{k}_{ch}")
            nc.sync.dma_start(
                out=t, in_=hT[k * P:(k + 1) * P, ch * CHW:(ch + 1) * CHW])
            ht_sb[k, ch] = t
        if ch < 3:
            for tp in range(3 * ch + 1, 3 * ch + 4):
                issue_mask("n", tp)

    # ---- fused main loop: Wh tiles then their attention work, per pair ----
    # One [P, F+1] accumulation chain per i-chunk: col F rides the ones
    # column of the wh operand, so the softmax denominator accumulates in
    # the same PSUM chain (one open accumulation group per bank).
    acc = [pp_acc.tile([P, F + 1], F32, tag=f"acc_{ic}", name=f"acc_{ic}")
           for ic in range(NIC)]
    s2ps = pp_acc.tile([P, 2 * NPR], F32, tag="s2ps", name="s2ps")

    # Persistent Wh operand buffers [Wh_h0 | 1 | Wh_h1 | 1]; the ones
    # columns are written once and survive manual slot reuse.
    WNB = DELAY + 3
    whb_sb = []
    for q in range(WNB):
        t = consts.tile([P, 2 * (F + 1)], BF16, tag=f"whb{q}")
        nc.vector.memset(t[:, F:F + 1], 1.0)
        nc.vector.memset(t[:, 2 * F + 1:2 * F + 2], 1.0)
        whb_sb.append(t)

    tb = {"n": [None] * NIC, "d": [None] * NIC}   # elu(o)+1 per (branch, ic)
    s1b = {}
    for b in BR:
        # s1 row vector, broadcast over partitions (per branch, just in time
        # so branch n's seed doesn't block produce on the in-order PE queue)
        ps1 = pp_work.tile([P, R], F32, tag="pswork")
        for k in range(NKT):
            nc.tensor.matmul(
                ps1, lhsT=wa_r[b][k], rhs=hto_sb[k],
                start=(k == 0), stop=(k == NKT - 1))
        s1t = consts.tile([P, 2 * R], BF16, tag=f"s1b_{b}")
        nc.scalar.copy(out=s1t[:, 0:R], in_=ps1)
        nc.vector.tensor_copy(out=s1t[:, R:2 * R], in_=ps1)
        s1b[b] = s1t

        def produce(tp):
            jt0 = 2 * tp
            ch, off = divmod(jt0 * P, CHW)
            if (b, tp) in mtiles:
                m = mtiles.pop((b, tp))
            else:
                issue_mask(b, tp)
                m = mtiles.pop((b, tp))
            lam = is_lam(tp)
            ump = workp.tile([P, 2 * R], BF16, tag="ump", name="ump", bufs=11)
            if not lam:
                # mask-add depends only on the mask DMA + s1b2: run it well
                # ahead of this pair's produce matmuls
                nc.vector.tensor_tensor(out=ump, in0=s1b[b], in1=m,
                                        op=ALU.add)
            ps = pp_work.tile([P, 2 * F], F32, tag="pswork", name="ps")
            for half in range(2):
                for k in range(NKT):
                    lhs = ht_sb[k, ch][:, off + half * P:off + (half + 1) * P]
                    nc.tensor.matmul(
                        ps[:, half * F:(half + 1) * F], lhsT=lhs,
                        rhs=waug_sb[b, k][:, 0:F],
                        start=(k == 0), stop=(k == NKT - 1))
                    nc.tensor.matmul(
                        s2ps[:, tp * 2 + half:tp * 2 + half + 1], lhsT=lhs,
                        rhs=waug_sb[b, k][:, F:F + 1],
                        start=(k == 0), stop=(k == NKT - 1))
            s2p = whp.tile([P, 2], F32, tag="s2", name="s2", bufs=14)
            nc.scalar.copy(out=s2p[:, 0:1], in_=s2ps[:, tp * 2:tp * 2 + 1])
            nc.scalar.copy(out=s2p[:, 1:2], in_=s2ps[:, tp * 2 + 1:tp * 2 + 2])
            s2s = [s2p[:, 0:1], s2p[:, 1:2]]
            whb = whb_sb[(tp + (0 if b == "n" else NPR)) % WNB]
            if lam:
                nc.scalar.copy(out=whb[:, 0:F], in_=ps[:, 0:F])
                nc.scalar.copy(out=whb[:, F + 1:2 * F + 1], in_=ps[:, F:2 * F])
            else:
                nc.vector.tensor_copy(out=whb[:, 0:F], in_=ps[:, 0:F])
                nc.vector.tensor_copy(out=whb[:, F + 1:2 * F + 1],
                                      in_=ps[:, F:2 * F])
            whs = [whb[:, 0:F + 1], whb[:, F + 1:2 * (F + 1)]]

            lrp = workp.tile([P, 2 * R], BF16, tag="lrp", name="lrp", bufs=8)
            if lam:
                # DVE-heavy scheme: stt logits + stt leaky-relu
                for half in range(2):
                    nc.vector.scalar_tensor_tensor(
                        out=ump[:, half * R:(half + 1) * R],
                        in0=s1b[b][:, half * R:(half + 1) * R],
                        scalar=s2s[half], in1=m[:, half * R:(half + 1) * R],
                        op0=ALU.add, op1=ALU.add)
                nc.vector.scalar_tensor_tensor(
                    out=lrp, in0=ump, scalar=ALPHA, in1=ump,
                    op0=ALU.mult, op1=ALU.max)
            else:
                # Prelu folds the per-j s2 bias and the leaky-relu into one
                # Act pass over the mask-add result
                for half in range(2):
                    nc.scalar.activation(
                        out=lrp[:, half * R:(half + 1) * R],
                        in_=ump[:, half * R:(half + 1) * R],
                        func=AF.Prelu, bias=s2s[half], alpha=ALPHA)
            ptp = pexp.tile([P, 2 * R], BF16, tag="ptp", name="ptp")
            nc.scalar.activation(out=ptp, in_=lrp, func=AF.Exp)
            return whs, ptp

        def consume(tp, whs, ptp):
            first, last = (tp == 0), (tp == NPR - 1)
            for half in range(2):
                pth = ptp[:, half * R:(half + 1) * R]
                st = first and half == 0
                sp = last and half == 1
                for ic in range(NIC):
                    lt = pth[:, ic * P:(ic + 1) * P]
                    nc.tensor.matmul(acc[ic], lhsT=lt, rhs=whs[half],
                                     start=st, stop=sp)

        inflight = []
        for tp in range(NPR):
            inflight.append((tp, *produce(tp)))
            if len(inflight) > DELAY:
                consume(*inflight.pop(0))
        for item in inflight:
            consume(*item)

        # per-branch epilogue: divide by row sums, elu (acc banks then free)
        rb = epip.tile([P, NIC], F32, tag="rb", name="rb", bufs=2)
        for ic in range(NIC):
            nc.vector.reciprocal(out=rb[:, ic:ic + 1],
                                 in_=acc[ic][:, F:F + 1])
        for ic in range(NIC):
            o = epip.tile([P, F], F32, tag="o", name="o", bufs=3)
            if ic % 2 == 0:
                nc.vector.tensor_scalar(out=o, in0=acc[ic][:, 0:F],
                                        scalar1=rb[:, ic:ic + 1], scalar2=None,
                                        op0=ALU.mult)
            else:
                nc.scalar.activation(out=o, in_=acc[ic][:, 0:F], func=AF.Copy,
                                     scale=rb[:, ic:ic + 1])
            rl = epip.tile([P, F], F32, tag="rl", name="rl", bufs=2)
            nc.vector.tensor_scalar_max(out=rl, in0=o, scalar1=0.0)
            em = epip.tile([P, F], F32, tag="em", name="em", bufs=2)
            nc.scalar.activation(out=em, in_=o, func=AF.Exp)
            t = epip.tile([P, F], F32, tag=f"t_{b}{ic}", name="t", bufs=1)
            # t = min(exp(o), 1) + relu(o)  ==  elu(o) + 1
            nc.vector.scalar_tensor_tensor(
                out=t, in0=em, scalar=1.0, in1=rl, op0=ALU.min, op1=ALU.add)
            tb[b][ic] = t

    for ic in range(NIC):
        c = epip.tile([P, F], F32, tag="comb", name="comb", bufs=4)
        # c = (t_n - 2) + t_d  ==  elu(o_n) + elu(o_d)
        nc.vector.scalar_tensor_tensor(
            out=c, in0=tb["n"][ic], scalar=-2.0, in1=tb["d"][ic],
            op0=ALU.add, op1=ALU.add)
        nc.sync.dma_start(out=OUT[ic * P:(ic + 1) * P, :], in_=c)


_CACHED = None


def _get_program():
    global _CACHED
    if _CACHED is None:
        _CACHED = build_program()
    return _CACHED


def _prep_inputs(h, adj_n, adj_d, W_n, a1_n, a2_n, W_d, a1_d, a2_d):
    h = np.asarray(h, np.float32)
    hT = np.ascontiguousarray(h.T).astype(ml_dtypes.bfloat16)
    com = {"ht": hT}
    for b, W, a1, a2 in (("n", W_n, a1_n, a2_n), ("d", W_d, a1_d, a2_d)):
        W = np.asarray(W, np.float32)
        waug = np.concatenate(
            [W, W @ np.asarray(a2, np.float32),
             W @ np.asarray(a1, np.float32)], axis=1)
        com[f"waug_{b}"] = waug.astype(ml_dtypes.bfloat16)
    adj = {"n": np.asarray(adj_n), "d": np.asarray(adj_d)}
    maps = []
    for c in range(NCORES):
        m = dict(com)
        m["hto"] = np.ascontiguousarray(hT[:, c * R:(c + 1) * R])
        for b in BR:
            blk = adj[b][c * R:(c + 1) * R, :]          # [R, N]
            mt = np.where(blk.T > 0, np.float32(0.0), np.float32(MASKB))
            m[f"mt_{b}"] = mt.astype(ml_dtypes.bfloat16)
            m[f"mt8_{b}"] = mt.astype(ml_dtypes.float8_e5m2)
        maps.append(m)
    return maps


def run_on_hw(inputs, trace=False):
    nc = _get_program()
    maps = _prep_inputs(
        inputs["h"], inputs["adj_n"], inputs["adj_d"],
        inputs["W_n"], inputs["a1_n"], inputs["a2_n"],
        inputs["W_d"], inputs["a1_d"], inputs["a2_d"])
    last_err = None
    for attempt in range(3):
        try:
            res = bass_utils.run_bass_kernel_spmd(
                nc, maps, core_ids=list(range(NCORES)), trace=trace)
            break
        except Exception as e:          # transient NRT/axon failures recover
            last_err = e
            import time as _time
            _time.sleep(5)
    else:
        raise last_err
    out = np.concatenate([res.results[c]["out"] for c in range(NCORES)],
                         axis=0)
    return out, res


def kernel(**inputs):
    out, _ = run_on_hw(inputs, trace=False)
    return out
